# revision 103
# baseline (speedup 1.0000x reference)
"""Trainium2 Bass kernel for an 8-head AttentionBlock (B=4, C=512, H=W=32).

Sharding: 8 cores; core c handles batch b=c//2, query half hf=c%2 (512 query
rows), all 8 heads. The k/v projection is computed for the full batch on both
cores of a pair so no cross-core communication is needed.

Structure (61us, from the 77.8us bf16 baseline; measured on HW):
 - q/k/v projections and attn@v run as fp8e4 DoubleRow matmuls: one
   instruction contracts TWO 128-row chunks in 216ns (2x PE throughput
   vs bf16; NOT the 4x the v2 cost model suggests). Weights are scaled
   x16 on the host so their N(0, 1/512) entries clear the fp8 subnormal
   range; the 1/16 folds into the psum->SBUF copies. fp8 DoubleRow for
   the OUT-projection was tried and produced scrambled results on HW
   (suspect walrus codegen for the [128,2,128] strided weight AP over a
   merged resT tile) - left bf16.
 - P = exp(scores*SCALE - 2.5) is stored fp8 via the SAME Schraudolph
   bit trick on both exp engines (scalar: activation Identity with
   scale/bias; DVE: one tensor_scalar), writing the fp8e4 bit pattern
   through a uint8 bitcast: f32->uint8 conversion rounds and saturates
   low at 0, which exactly implements flush-to-zero for tiny P. One
   formula everywhere means a softmax row never mixes exp methods, and
   the normalize cancels the systematic error (total rel err 7.1e-3 vs
   the 2e-2 gate, host-sim-verified before building).
 - Each scores unit is ONE [128,1024] psum tile = one key chunk x both
   heads of a pair, exp'd by ONE engine (18 scalar / 14 vector,
   Bresenham-spread). With the 4-deep unified psum pool this doubles
   the scores pipeline depth vs per-head tiles and removes the
   scalar/vector ping-pong.
 - Dependencies are TILE-granular: P is split into per-key-chunk-PAIR
   tiles so each attn@v matmul only waits on its own two exp writers;
   attn@v pairs weave into the scores stream as PE filler. The last
   pair's accumulation group is split so only its final two matmuls
   trail the last exp.
 - v8 tiles hold [ones(64) | v(64)] per head: attn@v psum rows 0:64 are
   64 copies of the denominator, so normalize is one [64,1024]
   reciprocal_approx_fast + two tensor_tensor mults per pair (no
   partition_broadcast, no single-partition copies).
 - The residual add runs on the PE as an identity-weight matmul inside
   each out-proj accumulation group; bias via epilogue (2 scalar +
   2 DVE, DMAs split across the sync and scalar rings). Epilogues are
   emitted after ALL matmuls: an epi read otherwise blocks the next
   matmul's write to the shared psum tile (tile-granular WAR).
 - ALL input DMAs ride the sync ring in strict priority order (DMA
   bandwidth is shared across rings; the gpsimd SW ring is slow):
   xf8a/wq8a/xf8b/wk8a first. Weight tensors are split so the first
   projections' blocks travel first. A dep-free dummy activation pulls
   ACT_TABLE_LOAD into the DMA window.
 - PE p-state matters: any PE idle resets the 2.4GHz clock to 1.2GHz
   for the next 3us of work, so every scores unit is paired with
   independent filler (projections, v casts, attn@v, out prestarts).
"""

import os
import sys
import types

sys.path.insert(0, "/opt/trn_rl_repo")


# Install the antenv.axon_hooks module if missing so NTFF profiling
# (trace=True / BASS_TRACE=1) works under axon.
def _install_axon_profile_hook():
    try:
        import antenv
    except ImportError:
        return
    if "antenv.axon_hooks" in sys.modules:
        return
    try:
        from antenv.axon_hooks import get_axon_ntff_profile_hook  # noqa: F401
        return  # real module exists
    except ImportError:
        pass
    mod = types.ModuleType("antenv.axon_hooks")
    mod._hook = None

    def set_axon_ntff_profile_hook(h):
        mod._hook = h

    def get_axon_ntff_profile_hook():
        return mod._hook

    mod.set_axon_ntff_profile_hook = set_axon_ntff_profile_hook
    mod.get_axon_ntff_profile_hook = get_axon_ntff_profile_hook
    sys.modules["antenv.axon_hooks"] = mod
    antenv.axon_hooks = mod
    try:
        from trn_agent_boot.trn_boot import _ntff_profile_via_ctypes

        so = "/opt/axon/libaxon_pjrt.so"
        if os.path.exists(so):
            set_axon_ntff_profile_hook(_ntff_profile_via_ctypes(so))
    except Exception:
        pass


_install_axon_profile_hook()

import numpy as np
from contextlib import ExitStack

import concourse.bass as bass  # noqa: F401
import concourse.bacc as bacc
import concourse.mybir as mybir
import concourse.tile as tile
from concourse.bass_utils import run_bass_kernel_spmd

F32 = mybir.dt.float32
BF16 = mybir.dt.bfloat16
F8 = mybir.dt.float8e4
U8 = mybir.dt.uint8
NP_BF16 = mybir.dt.np(BF16)
NP_F8 = mybir.dt.np(F8)
AF = mybir.ActivationFunctionType
ALU = mybir.AluOpType
PM = mybir.MatmulPerfMode

B, C, S = 4, 512, 1024  # batch, channels, spatial (H*W)
NH, DK = 8, 64
SCALE = DK ** -0.5
N_CORES = 8
SL = S // 2  # local query rows per core
WS = 16.0    # fp8 weight prescale

EXP_SHIFT = 2.5
# fp8e4m3 bits of e^y are ~ round(8/ln2 * y + 56); y = s*SCALE - EXP_SHIFT
EXP_A = float(8.0 / np.log(2.0) * SCALE)
EXP_B = float(56.0 - 8.0 / np.log(2.0) * EXP_SHIFT)


def _build():
    nc = bacc.Bacc("TRN2", target_bir_lowering=False, debug=False,
                   num_devices=N_CORES)

    # All DRAM tensors are [128, X] with contiguous per-partition rows so
    # every DMA is one contiguous block.
    # xf8[p, (kc2, i, s)] = x[c = kc2*256 + i*128 + p, s]  (hf-rotated s),
    # split by contraction pair so pair-0 matmuls start sooner
    xf8a_d = nc.dram_tensor("xf8a", [128, 2048], F8,
                            kind="ExternalInput").ap()
    xf8b_d = nc.dram_tensor("xf8b", [128, 2048], F8,
                            kind="ExternalInput").ap()
    # xbf[p, (cc, sl)] = x[c = cc*128 + p, local half]  (residual read)
    xbf_d = nc.dram_tensor("xbf", [128, 2048], BF16,
                           kind="ExternalInput").ap()
    # wq8/wk8[p, (hp, pair, i, m)] = 16*W.T[pair*256+i*128+p, hp*128+m],
    # hp=0 split out so the first projections' weights arrive first
    wq8a_d = nc.dram_tensor("wq8a", [128, 512], F8,
                            kind="ExternalInput").ap()
    wq8b_d = nc.dram_tensor("wq8b", [128, 1536], F8,
                            kind="ExternalInput").ap()
    wk8a_d = nc.dram_tensor("wk8a", [128, 512], F8,
                            kind="ExternalInput").ap()
    wk8b_d = nc.dram_tensor("wk8b", [128, 1536], F8,
                            kind="ExternalInput").ap()
    # wv8[p, (pair, i, f)] = 16*Wv.T[pair*256+i*128+p, f]
    wv8_d = nc.dram_tensor("wv8", [128, 2048], F8, kind="ExternalInput").ap()
    # wo[p, (hd, m)] = Wo.T[hd*128+p, m]
    wo_d = nc.dram_tensor("wo", [128, 2048], BF16, kind="ExternalInput").ap()
    ident_d = nc.dram_tensor("ident", [128, 128], BF16,
                             kind="ExternalInput").ap()
    # bpack columns: bq (4 chunks) | bo' (4 chunks), bo' = bo + Wo @ bv
    bp_d = nc.dram_tensor("bpack", [128, 8], F32, kind="ExternalInput").ap()
    # out rows [cc*128 .. +128) = out chunk cc, bf16 (host upcasts)
    out_d = nc.dram_tensor("out", [C, SL], BF16, kind="ExternalOutput").ap()

    with tile.TileContext(nc) as tc, ExitStack() as ctx:
        cst = ctx.enter_context(tc.tile_pool(name="cst", bufs=1))
        rpool = ctx.enter_context(tc.tile_pool(name="rp", bufs=4))
        opool = ctx.enter_context(tc.tile_pool(name="op", bufs=4))
        # PSUM: one shared 4-deep rotation of [128,1024] tiles (8 banks)
        # serving scores, projections, attn@v pairs AND the out-proj.
        psc = ctx.enter_context(tc.tile_pool(name="psc", bufs=4,
                                             space="PSUM"))

        # ---- persistent SBUF tiles ----
        xf8a_sb = cst.tile([128, 2048], F8, tag="xf8a", name="xf8a")
        xf8b_sb = cst.tile([128, 2048], F8, tag="xf8b", name="xf8b")
        xbf_sb = cst.tile([128, 2048], BF16, tag="xbf", name="xbf")
        wq8a_sb = cst.tile([128, 512], F8, tag="wq8a", name="wq8a")
        wq8b_sb = cst.tile([128, 1536], F8, tag="wq8b", name="wq8b")
        wk8a_sb = cst.tile([128, 512], F8, tag="wk8a", name="wk8a")
        wk8b_sb = cst.tile([128, 1536], F8, tag="wk8b", name="wk8b")
        wv8_sb = cst.tile([128, 2048], F8, tag="wv8", name="wv8")
        wo_sb = cst.tile([128, 2048], BF16, tag="wo", name="wo")
        id_sb = cst.tile([128, 128], BF16, tag="id", name="id")
        bp_sb = cst.tile([128, 8], F32, tag="bp", name="bp")
        ebias_sb = cst.tile([128, 1], F32, tag="eb", name="eb")
        qT = [cst.tile([128, SL], BF16, tag=f"qT{i}", name=f"qT{i}")
              for i in range(4)]
        kT = [cst.tile([128, S], BF16, tag=f"kT{i}", name=f"kT{i}")
              for i in range(4)]
        # v8[j][p, (i, h, e)]: key chunks 2j+i; e in [ones(64) | v(64)]
        v8 = [cst.tile([128, 2048], F8, tag=f"v8_{j}", name=f"v8_{j}")
              for j in range(4)]
        # P[hp][jj][p, (kc, hi, n)] fp8: one tile per kc PAIR so an attn@v
        # matmul only waits on its own two exp writers (deps are
        # tile-granular)
        P = [[cst.tile([128, 2048], F8, tag=f"P{hp}_{jj}",
                       name=f"P{hp}_{jj}") for jj in range(4)]
             for hp in range(4)]
        resT = [cst.tile([128, SL], BF16, tag=f"resT{i}", name=f"resT{i}")
                for i in range(4)]

        def wqv(hp, pair):  # wq8 [128, 2, 128] DoubleRow view
            sb = wq8a_sb if hp == 0 else wq8b_sb
            g = sb[:].rearrange("p (hp pr i m) -> p hp pr i m",
                                hp=(1 if hp == 0 else 3), pr=2, i=2)
            return g[:, hp if hp == 0 else hp - 1, pair]

        def wkv(hp, pair):
            sb = wk8a_sb if hp == 0 else wk8b_sb
            g = sb[:].rearrange("p (hp pr i m) -> p hp pr i m",
                                hp=(1 if hp == 0 else 3), pr=2, i=2)
            return g[:, hp if hp == 0 else hp - 1, pair]

        def wvv(pair):  # wv8 [128, 2, 512]
            g = wv8_sb[:].rearrange("p (pr i f) -> p pr i f", pr=2, i=2)
            return g[:, pair]

        def xv(pair, n0, n1):  # xf8 [128, 2, n1-n0]
            sb = xf8a_sb if pair == 0 else xf8b_sb
            g = sb[:].rearrange("p (i s) -> p i s", i=2)
            return g[:, :, n0:n1]

        def v8w(j, h):  # v8 weights [128, 2, 128] for head h, kc pair j
            g = v8[j][:].rearrange("p (i h e) -> p i h e", i=2, h=8)
            return g[:, :, h, :]

        def pview(hp, hi, j):  # P [128, 2, 512] moving view for kc pair j
            g = P[hp][j][:].rearrange("p (kc hi n) -> p kc hi n",
                                      kc=2, hi=2)
            return g[:, :, hi, :]

        # ---- input DMAs: ALL on the sync ring in strict priority order
        # (DMA bandwidth is shared across rings; serializing behind the
        # critical first blocks guarantees their priority).
        # first blocks issue on TWO rings in parallel (the per-DMA issue
        # slot is ~650ns; the scalar engine is free this early)
        nc.sync.dma_start(xf8a_sb[:], xf8a_d[:])
        nc.scalar.dma_start(wq8a_sb[:], wq8a_d[:])
        nc.scalar.dma_start(xf8b_sb[:], xf8b_d[:])
        nc.sync.dma_start(wk8a_sb[:], wk8a_d[:])
        nc.sync.dma_start(bp_sb[:], bp_d[:])
        nc.sync.dma_start(wq8b_sb[:], wq8b_d[:])
        nc.sync.dma_start(wv8_sb[:], wv8_d[:])
        nc.sync.dma_start(wk8b_sb[:], wk8b_d[:])
        nc.sync.dma_start(wo_sb[:], wo_d[:])
        nc.sync.dma_start(id_sb[:], ident_d[:])
        nc.sync.dma_start(xbf_sb[:], xbf_d[:])
        # ebias + a dep-free dummy activation so the scalar ACT table
        # loads during startup instead of blocking the first exp.
        nc.gpsimd.memset(ebias_sb[:], EXP_B)
        junk_sb = cst.tile([128, 1], F32, tag="junk", name="junk")
        nc.scalar.activation(junk_sb[:], ebias_sb[:], AF.Exp, scale=1.0)
        # ones columns in every v8 tile (written once, gpsimd)
        for j in range(4):
            g = v8[j][:].rearrange("p (i h e) -> p i h e", i=2, h=8)
            nc.gpsimd.memset(g[:, :, :, 0:64], 1.0)

        # ---- emit units ----
        def emit_q(hp):
            # qT[hp] = (16 Wq[hp] @ xs_local^T)/16 + bq; hp=0's copy runs
            # on the DVE (idle during the first units)
            ps = psc.tile([128, 1024], F32, tag="sc", name="sc")[:, 0:512]
            for pair in range(2):
                nc.tensor.matmul(ps, wqv(hp, pair), xv(pair, 0, SL),
                                 start=(pair == 0), stop=(pair == 1),
                                 perf_mode=PM.DoubleRow)
            if hp == 0:
                nc.vector.tensor_scalar(qT[hp][:], ps,
                                        1.0 / WS, bp_sb[:, hp:hp + 1],
                                        op0=ALU.mult, op1=ALU.add)
            else:
                nc.scalar.activation(qT[hp][:], ps, AF.Identity,
                                     scale=1.0 / WS,
                                     bias=bp_sb[:, hp:hp + 1])

        def emit_k_ns(hp, ns, ps):
            # one 512-key half of kT[hp]; no bias (cancels in softmax)
            for pair in range(2):
                nc.tensor.matmul(
                    ps[:, ns * 512:(ns + 1) * 512],
                    wkv(hp, pair), xv(pair, ns * 512, (ns + 1) * 512),
                    start=(pair == 0), stop=(pair == 1),
                    perf_mode=PM.DoubleRow)
            if hp == 0 and ns == 1:
                # DVE is idle in the first units; parallelize the
                # startup kT chain
                nc.vector.tensor_scalar(kT[hp][:, 512:1024],
                                        ps[:, 512:1024],
                                        1.0 / WS, None, op0=ALU.mult)
            else:
                nc.scalar.activation(kT[hp][:, ns * 512:(ns + 1) * 512],
                                     ps[:, ns * 512:(ns + 1) * 512],
                                     AF.Copy, scale=1.0 / WS)

        def emit_k(hp):
            ps = psc.tile([128, 1024], F32, tag="sc", name="sc")
            emit_k_ns(hp, 0, ps)
            emit_k_ns(hp, 1, ps)

        def emit_sc(hp, kc, eng):
            # scoresT [128 keys of chunk kc, 512 q] for BOTH heads of the
            # pair in one [128,1024] tile; ONE exp op on engine `eng`.
            # Both engines write the identical Schraudolph fp8 bit
            # pattern (f32->uint8 conversion rounds and saturates low to
            # +0), so a softmax row never mixes exp methods.
            ps = psc.tile([128, 1024], F32, tag="sc", name="sc")
            for hi in range(2):
                base = hi * 64
                nc.tensor.matmul(
                    ps[:, hi * SL:(hi + 1) * SL],
                    kT[hp][base:base + 64, kc * 128:(kc + 1) * 128],
                    qT[hp][base:base + 64, :],
                    start=True, stop=True,
                )
            pdst = P[hp][kc // 2][:, (kc % 2) * 1024:
                                  (kc % 2) * 1024 + 1024].bitcast(U8)
            if eng == "s":
                nc.scalar.activation(pdst, ps[:], AF.Identity,
                                     scale=EXP_A, bias=ebias_sb[:])
            else:
                nc.vector.tensor_scalar(
                    pdst, ps[:],
                    EXP_A, EXP_B, op0=ALU.mult, op1=ALU.add,
                )

        def emit_v(j, eng):
            # v rows for key chunks 2j, 2j+1 -> fp8 with 1/16 rescale;
            # the cast runs on `eng` (split across scalar+vector)
            ps = psc.tile([128, 1024], F32, tag="sc", name="sc")
            for i in range(2):
                rc = 2 * j + i
                for pair in range(2):
                    nc.tensor.matmul(
                        ps[:, i * 512:(i + 1) * 512],
                        xv(pair, rc * 128, (rc + 1) * 128), wvv(pair),
                        start=(pair == 0), stop=(pair == 1),
                        perf_mode=PM.DoubleRow)
            g = v8[j][:].rearrange("p (i h e) -> p i h e", i=2, h=8)
            if eng == "s":
                nc.scalar.activation(
                    g[:, :, :, 64:128],
                    ps[:].rearrange("p (i h e) -> p i h e", i=2, h=8),
                    AF.Copy, scale=1.0 / WS)
            else:
                nc.vector.tensor_scalar(
                    g[:, :, :, 64:128],
                    ps[:].rearrange("p (i h e) -> p i h e", i=2, h=8),
                    1.0 / WS, None, op0=ALU.mult)

        def emit_av2(h, pr, j0=0, j1=4):
            # attn @ [ones|v]: psum rows 0:64 = den (x64), 64:128 = res;
            # kc pairs [j0, j1) of the accumulation group
            hp, hi = h // 2, h % 2
            for j in range(j0, j1):
                nc.tensor.matmul(pr, v8w(j, h), pview(hp, hi, j),
                                 start=(j == 0), stop=(j == 3),
                                 perf_mode=PM.DoubleRow)

        def emit_norm_pair(hp, prt):
            # prt [128, 1024]: head 2hp in cols 0:512, 2hp+1 in 512:1024;
            # rows 0:64 = den (x64 copies), 64:128 = res
            rc_t = rpool.tile([64, 1024], F32, tag="rc", name="rc")
            nc.vector.reciprocal_approx_fast(rc_t[:], prt[0:64, :])
            for hi in range(2):
                nc.vector.tensor_tensor(
                    resT[hp][hi * 64:(hi + 1) * 64, :],
                    prt[64:128, hi * 512:(hi + 1) * 512],
                    rc_t[:, hi * 512:(hi + 1) * 512], op=ALU.mult,
                )

        def emit_out_mm(cc, ps, hd, start, stop):
            nc.tensor.matmul(
                ps,
                wo_sb[:, hd * 512 + cc * 128:hd * 512 + (cc + 1) * 128],
                resT[hd][:],
                start=start, stop=stop,
            )

        def emit_out_res(cc, ps):
            # residual: I @ xs chunk cc (mid-group)
            nc.tensor.matmul(ps, id_sb[:],
                             xbf_sb[:, cc * 512:(cc + 1) * 512],
                             start=False, stop=False)

        def emit_out_epi(cc, ps):
            # psum + bo' -> bf16; split across scalar/vector and both DMA
            # rings so the four epilogues pipeline at the tail
            ot = opool.tile([128, SL], BF16, tag="ob", name="ob")
            if cc % 2 == 0:
                nc.scalar.activation(ot[:], ps, AF.Identity,
                                     scale=1.0,
                                     bias=bp_sb[:, 4 + cc:5 + cc])
            else:
                nc.vector.tensor_scalar(ot[:], ps,
                                        bp_sb[:, 4 + cc:5 + cc], None,
                                        op0=ALU.add)
            q = nc.sync if cc < 2 else nc.scalar
            q.dma_start(out_d[cc * 128:(cc + 1) * 128, :], ot[:])

        # ---- woven emission schedule ----
        # Scores tiles stream to the two exp engines; every sc unit is
        # paired with independent PE filler (projections, v units, attn@v
        # pairs, out-proj prestarts) so the PE stays continuously busy
        # (idle resets the 2.4GHz p-state to 1.2GHz for 3us). v casts run
        # first (attn@v reads all four v8 tiles); attn@v pairs weave in
        # once their P tiles are a couple of units past their scores.
        av_pr = {}

        def av_pair_mm(hp, j1=4):
            prt = psc.tile([128, 1024], F32, tag="sc", name="sc")
            av_pr[hp] = prt
            emit_av2(2 * hp, prt[:, 0:512], 0, j1)

        def av_pair_mm2(hp):
            emit_av2(2 * hp + 1, av_pr[hp][:, 512:1024])

        def av_pair_mm2_31():
            emit_av2(7, av_pr[3][:, 512:1024], 0, 2)

        pso = {}

        def out_prestart(cc):
            ps = pso[cc]
            emit_out_mm(cc, ps, 1, True, False)
            emit_out_mm(cc, ps, 0, False, False)
            emit_out_res(cc, ps)

        # PE clock warmup: the PE idles ~7-10.7us waiting for input DMAs
        # and then pays the 1.2GHz p-state ramp on the first ~3us of real
        # matmuls. These dummy matmuls have NO input dependencies (warm_sb
        # is never written -> garbage data, result discarded when the
        # psum slot's next real user starts with start=True), so they
        # fire at preamble end and hand the real matmuls a 2.4GHz PE.
        warm_sb = cst.tile([128, 512], BF16, tag="warm", name="warm")
        nc.vector.memset(warm_sb[:], 0.0)  # DVE is idle until ~12.8us
        wps = psc.tile([128, 1024], F32, tag="sc", name="sc")
        for i in range(9):
            nc.tensor.matmul(wps[:, 0:512], warm_sb[:, 0:128],
                             warm_sb[:], start=True, stop=True)

        # prologue: q(0) and only the FIRST half of k(0) — the ns1 half
        # is deferred past the first scores unit's emission so sc(0,0)
        # doesn't wait on the (late, vector-queued) ns1 copy (readers
        # only wait on writers emitted before them)
        emit_q(0)
        kps0 = psc.tile([128, 1024], F32, tag="sc", name="sc")
        emit_k_ns(0, 0, kps0)

        def alloc_pso01():
            pso01 = psc.tile([128, 1024], F32, tag="sc", name="sc")
            pso[0] = pso01[:, 0:512]
            pso[1] = pso01[:, 512:1024]

        # filler per 1-based unit index (32 units of (hp, kc))
        fillers = {
            1: lambda: emit_k_ns(0, 1, kps0),
            2: lambda: emit_v(0, "v"),
            4: lambda: emit_v(1, "s"),
            5: lambda: emit_q(1),
            6: lambda: emit_k(1),
            8: lambda: emit_v(2, "v"),
            9: lambda: emit_v(3, "s"),
            12: lambda: av_pair_mm(0),
            13: lambda: emit_q(2),
            14: lambda: emit_k(2),
            15: lambda: av_pair_mm2(0),
            16: lambda: emit_norm_pair(0, av_pr[0][:]),
            20: lambda: av_pair_mm(1),
            21: lambda: emit_q(3),
            22: lambda: emit_k(3),
            23: lambda: av_pair_mm2(1),
            24: lambda: emit_norm_pair(1, av_pr[1][:]),
            26: lambda: av_pair_mm(2),
            28: lambda: (alloc_pso01(), out_prestart(0)),
            29: lambda: av_pair_mm2(2),
            30: lambda: (out_prestart(1),
                         emit_norm_pair(2, av_pr[2][:])),
            31: lambda: av_pair_mm(3, j1=2),
            32: lambda: av_pair_mm2_31(),
        }
        # exp engine per unit: 18 scalar / 14 vector, spread evenly
        u = 0
        for hp in range(4):
            for kc in range(8):
                u += 1
                eng = "s" if (u * 18) // 32 > ((u - 1) * 18) // 32 else "v"
                emit_sc(hp, kc, eng)
                if u in fillers:
                    fillers[u]()

        # tail: last attn@v pair + remaining out-proj; the cc2/cc3
        # prestarts run first so the PE has work while the last P tiles
        # finish their exp; hd3 matmuls wait only on resT[3]; epilogues
        # run after ALL matmuls so no epi read blocks a later matmul's
        # write to the shared psum tile.
        pso23 = psc.tile([128, 1024], F32, tag="sc", name="sc")
        pso[2] = pso23[:, 0:512]
        pso[3] = pso23[:, 512:1024]
        out_prestart(2)
        out_prestart(3)
        # finish the last pair's accumulation (kc pairs 2,3)
        emit_av2(6, av_pr[3][:, 0:512], 2, 4)
        emit_av2(7, av_pr[3][:, 512:1024], 2, 4)
        # per-head norm on the last pair: both recips first (head 7's
        # doesn't queue behind head 6's multiply)
        rc_l = []
        for hi in range(2):
            rc_t = rpool.tile([64, 512], F32, tag=f"rcl{hi}",
                              name=f"rcl{hi}")
            nc.vector.reciprocal_approx_fast(
                rc_t[:], av_pr[3][0:64, hi * 512:(hi + 1) * 512])
            rc_l.append(rc_t)
        for hi in range(2):
            nc.vector.tensor_tensor(
                resT[3][hi * 64:(hi + 1) * 64, :],
                av_pr[3][64:128, hi * 512:(hi + 1) * 512],
                rc_l[hi][:], op=ALU.mult,
            )
        for cc in range(4):
            emit_out_mm(cc, pso[cc], 2, False, False)
        for cc in range(4):
            emit_out_mm(cc, pso[cc], 3, False, True)
        for cc in range(4):
            emit_out_epi(cc, pso[cc])

    nc.compile()
    return nc


_NC_CACHE = None


def _get_nc():
    global _NC_CACHE
    if _NC_CACHE is None:
        _NC_CACHE = _build()
    return _NC_CACHE


def _prep_inputs(x, Wp, bp, Wo, bo):
    """Host-side reshape/reorder of weights; returns per-core input maps."""
    x = np.ascontiguousarray(x, dtype=np.float32)
    Wp = np.asarray(Wp, dtype=np.float32)
    bp = np.asarray(bp, dtype=np.float32)
    Wo = np.asarray(Wo, dtype=np.float32)
    bo = np.asarray(bo, dtype=np.float32)

    # Wp rows per head h: [h*192, h*192+64) = q, +64..128 = k, +128..192 = v
    Wp3 = Wp.reshape(NH, 3, DK, C)
    Wq = Wp3[:, 0].reshape(NH * DK, C)
    Wk = Wp3[:, 1].reshape(NH * DK, C)
    Wv = Wp3[:, 2].reshape(NH * DK, C)
    bp3 = bp.reshape(NH, 3, DK)
    bq = bp3[:, 0].reshape(-1)
    bv = bp3[:, 2].reshape(-1)
    # fold the v bias into the output projection bias (attn rows sum to 1)
    bo_eff = bo + Wo @ bv

    def pack_dr_w(WT, width):
        # WT [C, width] -> [128, (pair, i, width)]: 16*WT fp8 DoubleRow
        w = (WT * WS).astype(NP_F8)
        out = np.empty((128, 2, 2, width), dtype=NP_F8)
        for pair in range(2):
            for i in range(2):
                out[:, pair, i, :] = w[pair * 256 + i * 128:
                                       pair * 256 + i * 128 + 128, :]
        return np.ascontiguousarray(out.reshape(128, 4 * width))

    def pack_dr_whp(WT):
        # WT [C, 512] -> [128, (hp, pair, i, 128)]
        w = (WT * WS).astype(NP_F8)
        out = np.empty((128, 4, 2, 2, 128), dtype=NP_F8)
        for hp in range(4):
            for pair in range(2):
                for i in range(2):
                    out[:, hp, pair, i, :] = \
                        w[pair * 256 + i * 128:pair * 256 + i * 128 + 128,
                          hp * 128:(hp + 1) * 128]
        return np.ascontiguousarray(out.reshape(128, 2048))

    bpack = np.concatenate(
        [bq.reshape(4, 128).T, bo_eff.reshape(4, 128).T], axis=1)

    wq8 = pack_dr_whp(Wq.T)
    wk8 = pack_dr_whp(Wk.T)
    shared = {
        "wq8a": np.ascontiguousarray(wq8[:, 0:512]),
        "wq8b": np.ascontiguousarray(wq8[:, 512:2048]),
        "wk8a": np.ascontiguousarray(wk8[:, 0:512]),
        "wk8b": np.ascontiguousarray(wk8[:, 512:2048]),
        "wv8": pack_dr_w(Wv.T, 512),
        "wo": np.ascontiguousarray(Wo.T.reshape(4, 128, 512)
                                   .transpose(1, 0, 2).reshape(128, 2048)
                                   .astype(NP_BF16)),
        "ident": np.ascontiguousarray(np.eye(128, dtype=NP_BF16)),
        "bpack": np.ascontiguousarray(bpack.astype(np.float32)),
    }

    in_maps = []
    for c in range(N_CORES):
        b, hf = c // 2, c % 2
        xbc = x[b].reshape(C, S)
        if hf == 0:
            xs = xbc
        else:
            xs = np.concatenate([xbc[:, SL:], xbc[:, :SL]], axis=1)
        m = dict(shared)
        # xf8: [128, (i, s)] per contraction pair
        xf = xs.reshape(4, 128, S).astype(NP_F8)  # chunk-major
        xf = xf.transpose(1, 0, 2).reshape(128, 4096)
        m["xf8a"] = np.ascontiguousarray(xf[:, 0:2048])
        m["xf8b"] = np.ascontiguousarray(xf[:, 2048:4096])
        # xbf: [128, (cc, sl)] local half only
        xl = xs[:, 0:SL].reshape(4, 128, SL).astype(NP_BF16)
        m["xbf"] = np.ascontiguousarray(
            xl.transpose(1, 0, 2).reshape(128, 2048))
        in_maps.append(m)
    return in_maps


def _unshard(results):
    out = np.empty((B, C, S), dtype=np.float32)
    for c in range(N_CORES):
        b, hf = c // 2, c % 2
        out[b][:, hf * SL:(hf + 1) * SL] = \
            results[c]["out"].astype(np.float32)  # [C, SL]
    H = int(np.sqrt(S))
    return out.reshape(B, C, H, H)


def kernel(x, Wp, bp, Wo, bo):
    nc = _get_nc()
    in_maps = _prep_inputs(x, Wp, bp, Wo, bo)
    res = run_bass_kernel_spmd(nc, in_maps, list(range(N_CORES)))
    return _unshard(res.results)


# revision 106
# speedup vs baseline: 1.0489x; 1.0489x over previous
"""Trainium2 Bass kernel for an 8-head AttentionBlock (B=4, C=512, H=W=32).

Sharding: 8 cores; core c handles batch b=c//2, query half hf=c%2 (512 query
rows), all 8 heads. The k/v projection is computed for the full batch on both
cores of a pair so no cross-core communication is needed.

Structure (59.1-60.5us, from the 77.8us bf16 baseline; measured on HW.
Note the device has multi-minute throttled windows where all engines run
~15-20% slower and the same kernel measures ~70us):
 - PE clock warmup: 9 dependency-free dummy matmuls on a memset-once
   SBUF tile fire at engine-preamble end (~7.9us) and burn the
   1.2GHz->2.4GHz p-state ramp during the input-DMA wait, so the real
   matmul stream starts at full clock.
 - q/k/v projections and attn@v run as fp8e4 DoubleRow matmuls: one
   instruction contracts TWO 128-row chunks in 216ns (2x PE throughput
   vs bf16; NOT the 4x the v2 cost model suggests). Weights are scaled
   x16 on the host so their N(0, 1/512) entries clear the fp8 subnormal
   range; the 1/16 folds into the psum->SBUF copies. fp8 DoubleRow for
   the OUT-projection was tried and produced scrambled results on HW
   (suspect walrus codegen for the [128,2,128] strided weight AP over a
   merged resT tile) - left bf16.
 - P = exp(scores*SCALE - 2.5) is stored fp8 via the SAME Schraudolph
   bit trick on both exp engines (scalar: activation Identity with
   scale/bias; DVE: one tensor_scalar), writing the fp8e4 bit pattern
   through a uint8 bitcast: f32->uint8 conversion rounds and saturates
   low at 0, which exactly implements flush-to-zero for tiny P. One
   formula everywhere means a softmax row never mixes exp methods, and
   the normalize cancels the systematic error (total rel err 7.1e-3 vs
   the 2e-2 gate, host-sim-verified before building).
 - Each scores unit is ONE [128,1024] psum tile = one key chunk x both
   heads of a pair, exp'd by ONE engine (18 scalar / 14 vector,
   Bresenham-spread). With the 4-deep unified psum pool this doubles
   the scores pipeline depth vs per-head tiles and removes the
   scalar/vector ping-pong.
 - Dependencies are TILE-granular: P is split into per-key-chunk-PAIR
   tiles so each attn@v matmul only waits on its own two exp writers;
   attn@v pairs weave into the scores stream as PE filler. The last
   pair's accumulation group is split so only its final two matmuls
   trail the last exp.
 - v8 tiles hold [ones(64) | v(64)] per head: attn@v psum rows 0:64 are
   64 copies of the denominator, so normalize is one [64,1024]
   reciprocal_approx_fast + two tensor_tensor mults per pair (no
   partition_broadcast, no single-partition copies).
 - The residual add runs on the PE as an identity-weight matmul inside
   each out-proj accumulation group; bias via epilogue (2 scalar +
   2 DVE, DMAs split across the sync and scalar rings). Epilogues are
   emitted after ALL matmuls: an epi read otherwise blocks the next
   matmul's write to the shared psum tile (tile-granular WAR).
 - ALL input DMAs ride the sync ring in strict priority order (DMA
   bandwidth is shared across rings; the gpsimd SW ring is slow):
   xf8a/wq8a/xf8b/wk8a first. Weight tensors are split so the first
   projections' blocks travel first. A dep-free dummy activation pulls
   ACT_TABLE_LOAD into the DMA window.
 - PE p-state matters: any PE idle resets the 2.4GHz clock to 1.2GHz
   for the next 3us of work, so every scores unit is paired with
   independent filler (projections, v casts, attn@v, out prestarts).
"""

import os
import sys
import types

sys.path.insert(0, "/opt/trn_rl_repo")


# Install the antenv.axon_hooks module if missing so NTFF profiling
# (trace=True / BASS_TRACE=1) works under axon.
def _install_axon_profile_hook():
    try:
        import antenv
    except ImportError:
        return
    if "antenv.axon_hooks" in sys.modules:
        return
    try:
        from antenv.axon_hooks import get_axon_ntff_profile_hook  # noqa: F401
        return  # real module exists
    except ImportError:
        pass
    mod = types.ModuleType("antenv.axon_hooks")
    mod._hook = None

    def set_axon_ntff_profile_hook(h):
        mod._hook = h

    def get_axon_ntff_profile_hook():
        return mod._hook

    mod.set_axon_ntff_profile_hook = set_axon_ntff_profile_hook
    mod.get_axon_ntff_profile_hook = get_axon_ntff_profile_hook
    sys.modules["antenv.axon_hooks"] = mod
    antenv.axon_hooks = mod
    try:
        from trn_agent_boot.trn_boot import _ntff_profile_via_ctypes

        so = "/opt/axon/libaxon_pjrt.so"
        if os.path.exists(so):
            set_axon_ntff_profile_hook(_ntff_profile_via_ctypes(so))
    except Exception:
        pass


_install_axon_profile_hook()

import numpy as np
from contextlib import ExitStack

import concourse.bass as bass  # noqa: F401
import concourse.bacc as bacc
import concourse.mybir as mybir
import concourse.tile as tile
from concourse.bass_utils import run_bass_kernel_spmd

F32 = mybir.dt.float32
BF16 = mybir.dt.bfloat16
F8 = mybir.dt.float8e4
U8 = mybir.dt.uint8
NP_BF16 = mybir.dt.np(BF16)
NP_F8 = mybir.dt.np(F8)
AF = mybir.ActivationFunctionType
ALU = mybir.AluOpType
PM = mybir.MatmulPerfMode

B, C, S = 4, 512, 1024  # batch, channels, spatial (H*W)
NH, DK = 8, 64
SCALE = DK ** -0.5
N_CORES = 8
SL = S // 2  # local query rows per core
WS = 16.0    # fp8 weight prescale

EXP_SHIFT = 2.5
# fp8e4m3 bits of e^y are ~ round(8/ln2 * y + 56); y = s*SCALE - EXP_SHIFT
EXP_A = float(8.0 / np.log(2.0) * SCALE)
EXP_B = float(56.0 - 8.0 / np.log(2.0) * EXP_SHIFT)


def _build():
    nc = bacc.Bacc("TRN2", target_bir_lowering=False, debug=False,
                   num_devices=N_CORES)

    # All DRAM tensors are [128, X] with contiguous per-partition rows so
    # every DMA is one contiguous block.
    # xf8[p, (kc2, i, s)] = x[c = kc2*256 + i*128 + p, s]  (hf-rotated s),
    # split by contraction pair so pair-0 matmuls start sooner
    xf8a_d = nc.dram_tensor("xf8a", [128, 2048], F8,
                            kind="ExternalInput").ap()
    xf8b_d = nc.dram_tensor("xf8b", [128, 2048], F8,
                            kind="ExternalInput").ap()
    # xbf[p, (cc, sl)] = x[c = cc*128 + p, local half]  (residual read)
    xbf_d = nc.dram_tensor("xbf", [128, 2048], BF16,
                           kind="ExternalInput").ap()
    # wq8/wk8[p, (hp, pair, i, m)] = 16*W.T[pair*256+i*128+p, hp*128+m],
    # hp=0 split out so the first projections' weights arrive first
    wq8a_d = nc.dram_tensor("wq8a", [128, 512], F8,
                            kind="ExternalInput").ap()
    wq8b_d = nc.dram_tensor("wq8b", [128, 1536], F8,
                            kind="ExternalInput").ap()
    wk8a_d = nc.dram_tensor("wk8a", [128, 512], F8,
                            kind="ExternalInput").ap()
    wk8b_d = nc.dram_tensor("wk8b", [128, 1536], F8,
                            kind="ExternalInput").ap()
    # wv8[p, (pair, i, f)] = 16*Wv.T[pair*256+i*128+p, f]
    wv8_d = nc.dram_tensor("wv8", [128, 2048], F8, kind="ExternalInput").ap()
    # wo[p, (hd, m)] = Wo.T[hd*128+p, m]
    wo_d = nc.dram_tensor("wo", [128, 2048], BF16, kind="ExternalInput").ap()
    ident_d = nc.dram_tensor("ident", [128, 128], BF16,
                             kind="ExternalInput").ap()
    # bpack columns: bq (4 chunks) | bo' (4 chunks), bo' = bo + Wo @ bv
    bp_d = nc.dram_tensor("bpack", [128, 8], F32, kind="ExternalInput").ap()
    # out rows [cc*128 .. +128) = out chunk cc, bf16 (host upcasts)
    out_d = nc.dram_tensor("out", [C, SL], BF16, kind="ExternalOutput").ap()

    with tile.TileContext(nc) as tc, ExitStack() as ctx:
        cst = ctx.enter_context(tc.tile_pool(name="cst", bufs=1))
        rpool = ctx.enter_context(tc.tile_pool(name="rp", bufs=4))
        opool = ctx.enter_context(tc.tile_pool(name="op", bufs=4))
        # PSUM: one shared 4-deep rotation of [128,1024] tiles (8 banks)
        # serving scores, projections, attn@v pairs AND the out-proj.
        psc = ctx.enter_context(tc.tile_pool(name="psc", bufs=4,
                                             space="PSUM"))

        # ---- persistent SBUF tiles ----
        xf8a_sb = cst.tile([128, 2048], F8, tag="xf8a", name="xf8a")
        xf8b_sb = cst.tile([128, 2048], F8, tag="xf8b", name="xf8b")
        xbf_sb = cst.tile([128, 2048], BF16, tag="xbf", name="xbf")
        wq8a_sb = cst.tile([128, 512], F8, tag="wq8a", name="wq8a")
        wq8b_sb = cst.tile([128, 1536], F8, tag="wq8b", name="wq8b")
        wk8a_sb = cst.tile([128, 512], F8, tag="wk8a", name="wk8a")
        wk8b_sb = cst.tile([128, 1536], F8, tag="wk8b", name="wk8b")
        wv8_sb = cst.tile([128, 2048], F8, tag="wv8", name="wv8")
        wo_sb = cst.tile([128, 2048], BF16, tag="wo", name="wo")
        id_sb = cst.tile([128, 128], BF16, tag="id", name="id")
        bp_sb = cst.tile([128, 8], F32, tag="bp", name="bp")
        ebias_sb = cst.tile([128, 1], F32, tag="eb", name="eb")
        qT = [cst.tile([128, SL], BF16, tag=f"qT{i}", name=f"qT{i}")
              for i in range(4)]
        kT = [cst.tile([128, S], BF16, tag=f"kT{i}", name=f"kT{i}")
              for i in range(4)]
        # v8[j][p, (i, h, e)]: key chunks 2j+i; e in [ones(64) | v(64)]
        v8 = [cst.tile([128, 2048], F8, tag=f"v8_{j}", name=f"v8_{j}")
              for j in range(4)]
        # P[hp][jj][p, (kc, hi, n)] fp8: one tile per kc PAIR so an attn@v
        # matmul only waits on its own two exp writers (deps are
        # tile-granular)
        P = [[cst.tile([128, 2048], F8, tag=f"P{hp}_{jj}",
                       name=f"P{hp}_{jj}") for jj in range(4)]
             for hp in range(4)]
        resT = [cst.tile([128, SL], BF16, tag=f"resT{i}", name=f"resT{i}")
                for i in range(4)]

        def wqv(hp, pair):  # wq8 [128, 2, 128] DoubleRow view
            sb = wq8a_sb if hp == 0 else wq8b_sb
            g = sb[:].rearrange("p (hp pr i m) -> p hp pr i m",
                                hp=(1 if hp == 0 else 3), pr=2, i=2)
            return g[:, hp if hp == 0 else hp - 1, pair]

        def wkv(hp, pair):
            sb = wk8a_sb if hp == 0 else wk8b_sb
            g = sb[:].rearrange("p (hp pr i m) -> p hp pr i m",
                                hp=(1 if hp == 0 else 3), pr=2, i=2)
            return g[:, hp if hp == 0 else hp - 1, pair]

        def wvv(pair):  # wv8 [128, 2, 512]
            g = wv8_sb[:].rearrange("p (pr i f) -> p pr i f", pr=2, i=2)
            return g[:, pair]

        def xv(pair, n0, n1):  # xf8 [128, 2, n1-n0]
            sb = xf8a_sb if pair == 0 else xf8b_sb
            g = sb[:].rearrange("p (i s) -> p i s", i=2)
            return g[:, :, n0:n1]

        def v8w(j, h):  # v8 weights [128, 2, 128] for head h, kc pair j
            g = v8[j][:].rearrange("p (i h e) -> p i h e", i=2, h=8)
            return g[:, :, h, :]

        def pview(hp, hi, j):  # P [128, 2, 512] moving view for kc pair j
            g = P[hp][j][:].rearrange("p (kc hi n) -> p kc hi n",
                                      kc=2, hi=2)
            return g[:, :, hi, :]

        # ---- input DMAs: ALL on the sync ring in strict priority order
        # (DMA bandwidth is shared across rings; serializing behind the
        # critical first blocks guarantees their priority).
        # first blocks issue on TWO rings in parallel (the per-DMA issue
        # slot is ~650ns; the scalar engine is free this early)
        nc.sync.dma_start(xf8a_sb[:], xf8a_d[:])
        nc.scalar.dma_start(wq8a_sb[:], wq8a_d[:])
        nc.scalar.dma_start(xf8b_sb[:], xf8b_d[:])
        nc.sync.dma_start(wk8a_sb[:], wk8a_d[:])
        nc.sync.dma_start(bp_sb[:], bp_d[:])
        nc.sync.dma_start(wq8b_sb[:], wq8b_d[:])
        nc.sync.dma_start(wv8_sb[:], wv8_d[:])
        nc.sync.dma_start(wk8b_sb[:], wk8b_d[:])
        nc.sync.dma_start(wo_sb[:], wo_d[:])
        nc.sync.dma_start(id_sb[:], ident_d[:])
        nc.sync.dma_start(xbf_sb[:], xbf_d[:])
        # ebias + a dep-free dummy activation so the scalar ACT table
        # loads during startup instead of blocking the first exp.
        nc.gpsimd.memset(ebias_sb[:], EXP_B)
        junk_sb = cst.tile([128, 1], F32, tag="junk", name="junk")
        nc.scalar.activation(junk_sb[:], ebias_sb[:], AF.Exp, scale=1.0)
        # ones columns in every v8 tile (written once, gpsimd)
        for j in range(4):
            g = v8[j][:].rearrange("p (i h e) -> p i h e", i=2, h=8)
            nc.gpsimd.memset(g[:, :, :, 0:64], 1.0)

        # ---- emit units ----
        def emit_q(hp):
            # qT[hp] = (16 Wq[hp] @ xs_local^T)/16 + bq; hp=0's copy runs
            # on the DVE (idle during the first units)
            ps = psc.tile([128, 1024], F32, tag="sc", name="sc")[:, 0:512]
            for pair in range(2):
                nc.tensor.matmul(ps, wqv(hp, pair), xv(pair, 0, SL),
                                 start=(pair == 0), stop=(pair == 1),
                                 perf_mode=PM.DoubleRow)
            if hp == 0:
                nc.vector.tensor_scalar(qT[hp][:], ps,
                                        1.0 / WS, bp_sb[:, hp:hp + 1],
                                        op0=ALU.mult, op1=ALU.add)
            else:
                nc.scalar.activation(qT[hp][:], ps, AF.Identity,
                                     scale=1.0 / WS,
                                     bias=bp_sb[:, hp:hp + 1])

        def emit_k_ns(hp, ns, ps):
            # one 512-key half of kT[hp]; no bias (cancels in softmax)
            for pair in range(2):
                nc.tensor.matmul(
                    ps[:, ns * 512:(ns + 1) * 512],
                    wkv(hp, pair), xv(pair, ns * 512, (ns + 1) * 512),
                    start=(pair == 0), stop=(pair == 1),
                    perf_mode=PM.DoubleRow)
            if hp == 0 and ns == 1:
                # DVE is idle in the first units; parallelize the
                # startup kT chain
                nc.vector.tensor_scalar(kT[hp][:, 512:1024],
                                        ps[:, 512:1024],
                                        1.0 / WS, None, op0=ALU.mult)
            else:
                nc.scalar.activation(kT[hp][:, ns * 512:(ns + 1) * 512],
                                     ps[:, ns * 512:(ns + 1) * 512],
                                     AF.Copy, scale=1.0 / WS)

        def emit_k(hp):
            ps = psc.tile([128, 1024], F32, tag="sc", name="sc")
            emit_k_ns(hp, 0, ps)
            emit_k_ns(hp, 1, ps)

        def emit_sc(hp, kc, eng):
            # scoresT [128 keys of chunk kc, 512 q] for BOTH heads of the
            # pair in one [128,1024] tile; ONE exp op on engine `eng`.
            # Both engines write the identical Schraudolph fp8 bit
            # pattern (f32->uint8 conversion rounds and saturates low to
            # +0), so a softmax row never mixes exp methods.
            ps = psc.tile([128, 1024], F32, tag="sc", name="sc")
            for hi in range(2):
                base = hi * 64
                nc.tensor.matmul(
                    ps[:, hi * SL:(hi + 1) * SL],
                    kT[hp][base:base + 64, kc * 128:(kc + 1) * 128],
                    qT[hp][base:base + 64, :],
                    start=True, stop=True,
                )
            pdst = P[hp][kc // 2][:, (kc % 2) * 1024:
                                  (kc % 2) * 1024 + 1024].bitcast(U8)
            if eng == "s":
                nc.scalar.activation(pdst, ps[:], AF.Identity,
                                     scale=EXP_A, bias=ebias_sb[:])
            else:
                nc.vector.tensor_scalar(
                    pdst, ps[:],
                    EXP_A, EXP_B, op0=ALU.mult, op1=ALU.add,
                )

        def emit_v(j, eng):
            # v rows for key chunks 2j, 2j+1 -> fp8 with 1/16 rescale;
            # the cast runs on `eng` (split across scalar+vector)
            ps = psc.tile([128, 1024], F32, tag="sc", name="sc")
            for i in range(2):
                rc = 2 * j + i
                for pair in range(2):
                    nc.tensor.matmul(
                        ps[:, i * 512:(i + 1) * 512],
                        xv(pair, rc * 128, (rc + 1) * 128), wvv(pair),
                        start=(pair == 0), stop=(pair == 1),
                        perf_mode=PM.DoubleRow)
            g = v8[j][:].rearrange("p (i h e) -> p i h e", i=2, h=8)
            if eng == "s":
                nc.scalar.activation(
                    g[:, :, :, 64:128],
                    ps[:].rearrange("p (i h e) -> p i h e", i=2, h=8),
                    AF.Copy, scale=1.0 / WS)
            else:
                nc.vector.tensor_scalar(
                    g[:, :, :, 64:128],
                    ps[:].rearrange("p (i h e) -> p i h e", i=2, h=8),
                    1.0 / WS, None, op0=ALU.mult)

        def emit_av2(h, pr, j0=0, j1=4):
            # attn @ [ones|v]: psum rows 0:64 = den (x64), 64:128 = res;
            # kc pairs [j0, j1) of the accumulation group
            hp, hi = h // 2, h % 2
            for j in range(j0, j1):
                nc.tensor.matmul(pr, v8w(j, h), pview(hp, hi, j),
                                 start=(j == 0), stop=(j == 3),
                                 perf_mode=PM.DoubleRow)

        def emit_norm_pair(hp, prt):
            # prt [128, 1024]: head 2hp in cols 0:512, 2hp+1 in 512:1024;
            # rows 0:64 = den (x64 copies), 64:128 = res
            rc_t = rpool.tile([64, 1024], F32, tag="rc", name="rc")
            nc.vector.reciprocal_approx_fast(rc_t[:], prt[0:64, :])
            for hi in range(2):
                nc.vector.tensor_tensor(
                    resT[hp][hi * 64:(hi + 1) * 64, :],
                    prt[64:128, hi * 512:(hi + 1) * 512],
                    rc_t[:, hi * 512:(hi + 1) * 512], op=ALU.mult,
                )

        def emit_out_mm(cc, ps, hd, start, stop):
            nc.tensor.matmul(
                ps,
                wo_sb[:, hd * 512 + cc * 128:hd * 512 + (cc + 1) * 128],
                resT[hd][:],
                start=start, stop=stop,
            )

        def emit_out_res(cc, ps):
            # residual: I @ xs chunk cc (mid-group)
            nc.tensor.matmul(ps, id_sb[:],
                             xbf_sb[:, cc * 512:(cc + 1) * 512],
                             start=False, stop=False)

        def emit_out_epi(cc, ps):
            # psum + bo' -> bf16; split across scalar/vector and both DMA
            # rings so the four epilogues pipeline at the tail
            ot = opool.tile([128, SL], BF16, tag="ob", name="ob")
            if cc % 2 == 0:
                nc.scalar.activation(ot[:], ps, AF.Identity,
                                     scale=1.0,
                                     bias=bp_sb[:, 4 + cc:5 + cc])
            else:
                nc.vector.tensor_scalar(ot[:], ps,
                                        bp_sb[:, 4 + cc:5 + cc], None,
                                        op0=ALU.add)
            q = nc.sync if cc < 2 else nc.scalar
            q.dma_start(out_d[cc * 128:(cc + 1) * 128, :], ot[:])

        # ---- woven emission schedule ----
        # Scores tiles stream to the two exp engines; every sc unit is
        # paired with independent PE filler (projections, v units, attn@v
        # pairs, out-proj prestarts) so the PE stays continuously busy
        # (idle resets the 2.4GHz p-state to 1.2GHz for 3us). v casts run
        # first (attn@v reads all four v8 tiles); attn@v pairs weave in
        # once their P tiles are a couple of units past their scores.
        av_pr = {}

        def av_pair_mm(hp, j1=4):
            prt = psc.tile([128, 1024], F32, tag="sc", name="sc")
            av_pr[hp] = prt
            emit_av2(2 * hp, prt[:, 0:512], 0, j1)

        def av_pair_mm2(hp):
            emit_av2(2 * hp + 1, av_pr[hp][:, 512:1024])

        def av_pair_mm2_31():
            emit_av2(7, av_pr[3][:, 512:1024], 0, 2)

        pso = {}

        def out_prestart(cc):
            ps = pso[cc]
            emit_out_mm(cc, ps, 1, True, False)
            emit_out_mm(cc, ps, 0, False, False)
            emit_out_res(cc, ps)

        # PE clock warmup: the PE idles ~7-10.7us waiting for input DMAs
        # and then pays the 1.2GHz p-state ramp on the first ~3us of real
        # matmuls. These dummy matmuls have NO input dependencies (warm_sb
        # is never written -> garbage data, result discarded when the
        # psum slot's next real user starts with start=True), so they
        # fire at preamble end and hand the real matmuls a 2.4GHz PE.
        warm_sb = cst.tile([128, 512], BF16, tag="warm", name="warm")
        nc.vector.memset(warm_sb[:], 0.0)  # DVE is idle until ~12.8us
        wps = psc.tile([128, 1024], F32, tag="sc", name="sc")
        for i in range(9):
            nc.tensor.matmul(wps[:, 0:512], warm_sb[:, 0:128],
                             warm_sb[:], start=True, stop=True)

        # prologue: q(0) and the FIRST half of k(0), with their pair-0
        # matmuls interleaved before the pair-1 ones so the PE has work
        # (k's pair-0 needs only the early wk8a+xf8a) while the xf8b DMA
        # is still in flight. The k ns1 half is deferred past the first
        # scores unit's emission so sc(0,0) doesn't wait on the (late,
        # vector-queued) ns1 copy (readers only wait on prior writers).
        qps0 = psc.tile([128, 1024], F32, tag="sc", name="sc")[:, 0:512]
        kps0 = psc.tile([128, 1024], F32, tag="sc", name="sc")
        nc.tensor.matmul(qps0, wqv(0, 0), xv(0, 0, SL),
                         start=True, stop=False, perf_mode=PM.DoubleRow)
        nc.tensor.matmul(kps0[:, 0:512], wkv(0, 0), xv(0, 0, 512),
                         start=True, stop=False, perf_mode=PM.DoubleRow)
        nc.tensor.matmul(qps0, wqv(0, 1), xv(1, 0, SL),
                         start=False, stop=True, perf_mode=PM.DoubleRow)
        nc.tensor.matmul(kps0[:, 0:512], wkv(0, 1), xv(1, 0, 512),
                         start=False, stop=True, perf_mode=PM.DoubleRow)
        nc.vector.tensor_scalar(qT[0][:], qps0, 1.0 / WS, bp_sb[:, 0:1],
                                op0=ALU.mult, op1=ALU.add)
        nc.scalar.activation(kT[0][:, 0:512], kps0[:, 0:512],
                             AF.Copy, scale=1.0 / WS)

        def alloc_pso01():
            pso01 = psc.tile([128, 1024], F32, tag="sc", name="sc")
            pso[0] = pso01[:, 0:512]
            pso[1] = pso01[:, 512:1024]

        # filler per 1-based unit index (32 units of (hp, kc))
        fillers = {
            1: lambda: emit_k_ns(0, 1, kps0),
            2: lambda: emit_v(0, "v"),
            4: lambda: emit_v(1, "s"),
            5: lambda: emit_q(1),
            6: lambda: emit_k(1),
            8: lambda: emit_v(2, "v"),
            9: lambda: emit_v(3, "s"),
            12: lambda: av_pair_mm(0),
            13: lambda: emit_q(2),
            14: lambda: emit_k(2),
            15: lambda: av_pair_mm2(0),
            16: lambda: emit_norm_pair(0, av_pr[0][:]),
            20: lambda: av_pair_mm(1),
            21: lambda: emit_q(3),
            22: lambda: emit_k(3),
            23: lambda: av_pair_mm2(1),
            24: lambda: emit_norm_pair(1, av_pr[1][:]),
            26: lambda: av_pair_mm(2),
            28: lambda: (alloc_pso01(), out_prestart(0)),
            29: lambda: av_pair_mm2(2),
            30: lambda: (out_prestart(1),
                         emit_norm_pair(2, av_pr[2][:])),
            31: lambda: av_pair_mm(3, j1=2),
            32: lambda: av_pair_mm2_31(),
        }
        # exp engine per unit: 18 scalar / 14 vector, spread evenly
        u = 0
        for hp in range(4):
            for kc in range(8):
                u += 1
                eng = "s" if (u * 18) // 32 > ((u - 1) * 18) // 32 else "v"
                emit_sc(hp, kc, eng)
                if u in fillers:
                    fillers[u]()

        # tail: last attn@v pair + remaining out-proj; the cc2/cc3
        # prestarts run first so the PE has work while the last P tiles
        # finish their exp; hd3 matmuls wait only on resT[3]; epilogues
        # run after ALL matmuls so no epi read blocks a later matmul's
        # write to the shared psum tile.
        pso23 = psc.tile([128, 1024], F32, tag="sc", name="sc")
        pso[2] = pso23[:, 0:512]
        pso[3] = pso23[:, 512:1024]
        out_prestart(2)
        out_prestart(3)
        # finish the last pair's accumulation (kc pairs 2,3)
        emit_av2(6, av_pr[3][:, 0:512], 2, 4)
        emit_av2(7, av_pr[3][:, 512:1024], 2, 4)
        # per-head norm on the last pair: both recips first (head 7's
        # doesn't queue behind head 6's multiply)
        rc_l = []
        for hi in range(2):
            rc_t = rpool.tile([64, 512], F32, tag=f"rcl{hi}",
                              name=f"rcl{hi}")
            nc.vector.reciprocal_approx_fast(
                rc_t[:], av_pr[3][0:64, hi * 512:(hi + 1) * 512])
            rc_l.append(rc_t)
        for hi in range(2):
            nc.vector.tensor_tensor(
                resT[3][hi * 64:(hi + 1) * 64, :],
                av_pr[3][64:128, hi * 512:(hi + 1) * 512],
                rc_l[hi][:], op=ALU.mult,
            )
        for cc in range(4):
            emit_out_mm(cc, pso[cc], 2, False, False)
        # epilogues fire as soon as their shared psum tile's LAST writer
        # (hd3 of the other cc half) is emitted
        for cc in range(2):
            emit_out_mm(cc, pso[cc], 3, False, True)
        emit_out_epi(0, pso[0])
        emit_out_epi(1, pso[1])
        for cc in range(2, 4):
            emit_out_mm(cc, pso[cc], 3, False, True)
        emit_out_epi(2, pso[2])
        emit_out_epi(3, pso[3])

    nc.compile()
    return nc


_NC_CACHE = None


def _get_nc():
    global _NC_CACHE
    if _NC_CACHE is None:
        _NC_CACHE = _build()
    return _NC_CACHE


def _prep_inputs(x, Wp, bp, Wo, bo):
    """Host-side reshape/reorder of weights; returns per-core input maps."""
    x = np.ascontiguousarray(x, dtype=np.float32)
    Wp = np.asarray(Wp, dtype=np.float32)
    bp = np.asarray(bp, dtype=np.float32)
    Wo = np.asarray(Wo, dtype=np.float32)
    bo = np.asarray(bo, dtype=np.float32)

    # Wp rows per head h: [h*192, h*192+64) = q, +64..128 = k, +128..192 = v
    Wp3 = Wp.reshape(NH, 3, DK, C)
    Wq = Wp3[:, 0].reshape(NH * DK, C)
    Wk = Wp3[:, 1].reshape(NH * DK, C)
    Wv = Wp3[:, 2].reshape(NH * DK, C)
    bp3 = bp.reshape(NH, 3, DK)
    bq = bp3[:, 0].reshape(-1)
    bv = bp3[:, 2].reshape(-1)
    # fold the v bias into the output projection bias (attn rows sum to 1)
    bo_eff = bo + Wo @ bv

    def pack_dr_w(WT, width):
        # WT [C, width] -> [128, (pair, i, width)]: 16*WT fp8 DoubleRow
        w = (WT * WS).astype(NP_F8)
        out = np.empty((128, 2, 2, width), dtype=NP_F8)
        for pair in range(2):
            for i in range(2):
                out[:, pair, i, :] = w[pair * 256 + i * 128:
                                       pair * 256 + i * 128 + 128, :]
        return np.ascontiguousarray(out.reshape(128, 4 * width))

    def pack_dr_whp(WT):
        # WT [C, 512] -> [128, (hp, pair, i, 128)]
        w = (WT * WS).astype(NP_F8)
        out = np.empty((128, 4, 2, 2, 128), dtype=NP_F8)
        for hp in range(4):
            for pair in range(2):
                for i in range(2):
                    out[:, hp, pair, i, :] = \
                        w[pair * 256 + i * 128:pair * 256 + i * 128 + 128,
                          hp * 128:(hp + 1) * 128]
        return np.ascontiguousarray(out.reshape(128, 2048))

    bpack = np.concatenate(
        [bq.reshape(4, 128).T, bo_eff.reshape(4, 128).T], axis=1)

    wq8 = pack_dr_whp(Wq.T)
    wk8 = pack_dr_whp(Wk.T)
    shared = {
        "wq8a": np.ascontiguousarray(wq8[:, 0:512]),
        "wq8b": np.ascontiguousarray(wq8[:, 512:2048]),
        "wk8a": np.ascontiguousarray(wk8[:, 0:512]),
        "wk8b": np.ascontiguousarray(wk8[:, 512:2048]),
        "wv8": pack_dr_w(Wv.T, 512),
        "wo": np.ascontiguousarray(Wo.T.reshape(4, 128, 512)
                                   .transpose(1, 0, 2).reshape(128, 2048)
                                   .astype(NP_BF16)),
        "ident": np.ascontiguousarray(np.eye(128, dtype=NP_BF16)),
        "bpack": np.ascontiguousarray(bpack.astype(np.float32)),
    }

    in_maps = []
    for c in range(N_CORES):
        b, hf = c // 2, c % 2
        xbc = x[b].reshape(C, S)
        if hf == 0:
            xs = xbc
        else:
            xs = np.concatenate([xbc[:, SL:], xbc[:, :SL]], axis=1)
        m = dict(shared)
        # xf8: [128, (i, s)] per contraction pair
        xf = xs.reshape(4, 128, S).astype(NP_F8)  # chunk-major
        xf = xf.transpose(1, 0, 2).reshape(128, 4096)
        m["xf8a"] = np.ascontiguousarray(xf[:, 0:2048])
        m["xf8b"] = np.ascontiguousarray(xf[:, 2048:4096])
        # xbf: [128, (cc, sl)] local half only
        xl = xs[:, 0:SL].reshape(4, 128, SL).astype(NP_BF16)
        m["xbf"] = np.ascontiguousarray(
            xl.transpose(1, 0, 2).reshape(128, 2048))
        in_maps.append(m)
    return in_maps


def _unshard(results):
    out = np.empty((B, C, S), dtype=np.float32)
    for c in range(N_CORES):
        b, hf = c // 2, c % 2
        out[b][:, hf * SL:(hf + 1) * SL] = \
            results[c]["out"].astype(np.float32)  # [C, SL]
    H = int(np.sqrt(S))
    return out.reshape(B, C, H, H)


def kernel(x, Wp, bp, Wo, bo):
    nc = _get_nc()
    in_maps = _prep_inputs(x, Wp, bp, Wo, bo)
    res = run_bass_kernel_spmd(nc, in_maps, list(range(N_CORES)))
    return _unshard(res.results)


# revision 107
# speedup vs baseline: 1.0695x; 1.0196x over previous
"""Trainium2 Bass kernel for an 8-head AttentionBlock (B=4, C=512, H=W=32).

Sharding: 8 cores; core c handles batch b=c//2, query half hf=c%2 (512 query
rows), all 8 heads. The k/v projection is computed for the full batch on both
cores of a pair so no cross-core communication is needed.

Structure (59.1-60.5us, from the 77.8us bf16 baseline; measured on HW.
Note the device has multi-minute throttled windows where all engines run
~15-20% slower and the same kernel measures ~70us):
 - PE clock warmup: 9 dependency-free dummy matmuls on a memset-once
   SBUF tile fire at engine-preamble end (~7.9us) and burn the
   1.2GHz->2.4GHz p-state ramp during the input-DMA wait, so the real
   matmul stream starts at full clock.
 - q/k/v projections and attn@v run as fp8e4 DoubleRow matmuls: one
   instruction contracts TWO 128-row chunks in 216ns (2x PE throughput
   vs bf16; NOT the 4x the v2 cost model suggests). Weights are scaled
   x16 on the host so their N(0, 1/512) entries clear the fp8 subnormal
   range; the 1/16 folds into the psum->SBUF copies. fp8 DoubleRow for
   the OUT-projection was tried and produced scrambled results on HW
   (suspect walrus codegen for the [128,2,128] strided weight AP over a
   merged resT tile) - left bf16.
 - P = exp(scores*SCALE - 2.5) is stored fp8 via the SAME Schraudolph
   bit trick on both exp engines (scalar: activation Identity with
   scale/bias; DVE: one tensor_scalar), writing the fp8e4 bit pattern
   through a uint8 bitcast: f32->uint8 conversion rounds and saturates
   low at 0, which exactly implements flush-to-zero for tiny P. One
   formula everywhere means a softmax row never mixes exp methods, and
   the normalize cancels the systematic error (total rel err 7.1e-3 vs
   the 2e-2 gate, host-sim-verified before building).
 - Each scores unit is ONE [128,1024] psum tile = one key chunk x both
   heads of a pair, exp'd by ONE engine (18 scalar / 14 vector,
   Bresenham-spread). With the 4-deep unified psum pool this doubles
   the scores pipeline depth vs per-head tiles and removes the
   scalar/vector ping-pong.
 - Dependencies are TILE-granular: P is split into per-key-chunk-PAIR
   tiles so each attn@v matmul only waits on its own two exp writers;
   attn@v pairs weave into the scores stream as PE filler. The last
   pair's accumulation group is split so only its final two matmuls
   trail the last exp.
 - v8 tiles hold [ones(64) | v(64)] per head: attn@v psum rows 0:64 are
   64 copies of the denominator, so normalize is one [64,1024]
   reciprocal_approx_fast + two tensor_tensor mults per pair (no
   partition_broadcast, no single-partition copies).
 - The residual add runs on the PE as an identity-weight matmul inside
   each out-proj accumulation group; bias via epilogue (2 scalar +
   2 DVE, DMAs split across the sync and scalar rings). Epilogues are
   emitted after ALL matmuls: an epi read otherwise blocks the next
   matmul's write to the shared psum tile (tile-granular WAR).
 - ALL input DMAs ride the sync ring in strict priority order (DMA
   bandwidth is shared across rings; the gpsimd SW ring is slow):
   xf8a/wq8a/xf8b/wk8a first. Weight tensors are split so the first
   projections' blocks travel first. A dep-free dummy activation pulls
   ACT_TABLE_LOAD into the DMA window.
 - PE p-state matters: any PE idle resets the 2.4GHz clock to 1.2GHz
   for the next 3us of work, so every scores unit is paired with
   independent filler (projections, v casts, attn@v, out prestarts).
"""

import os
import sys
import types

sys.path.insert(0, "/opt/trn_rl_repo")


# Install the antenv.axon_hooks module if missing so NTFF profiling
# (trace=True / BASS_TRACE=1) works under axon.
def _install_axon_profile_hook():
    try:
        import antenv
    except ImportError:
        return
    if "antenv.axon_hooks" in sys.modules:
        return
    try:
        from antenv.axon_hooks import get_axon_ntff_profile_hook  # noqa: F401
        return  # real module exists
    except ImportError:
        pass
    mod = types.ModuleType("antenv.axon_hooks")
    mod._hook = None

    def set_axon_ntff_profile_hook(h):
        mod._hook = h

    def get_axon_ntff_profile_hook():
        return mod._hook

    mod.set_axon_ntff_profile_hook = set_axon_ntff_profile_hook
    mod.get_axon_ntff_profile_hook = get_axon_ntff_profile_hook
    sys.modules["antenv.axon_hooks"] = mod
    antenv.axon_hooks = mod
    try:
        from trn_agent_boot.trn_boot import _ntff_profile_via_ctypes

        so = "/opt/axon/libaxon_pjrt.so"
        if os.path.exists(so):
            set_axon_ntff_profile_hook(_ntff_profile_via_ctypes(so))
    except Exception:
        pass


_install_axon_profile_hook()

import numpy as np
from contextlib import ExitStack

import concourse.bass as bass  # noqa: F401
import concourse.bacc as bacc
import concourse.mybir as mybir
import concourse.tile as tile
from concourse.bass_utils import run_bass_kernel_spmd

F32 = mybir.dt.float32
BF16 = mybir.dt.bfloat16
F8 = mybir.dt.float8e4
U8 = mybir.dt.uint8
NP_BF16 = mybir.dt.np(BF16)
NP_F8 = mybir.dt.np(F8)
AF = mybir.ActivationFunctionType
ALU = mybir.AluOpType
PM = mybir.MatmulPerfMode

B, C, S = 4, 512, 1024  # batch, channels, spatial (H*W)
NH, DK = 8, 64
SCALE = DK ** -0.5
N_CORES = 8
SL = S // 2  # local query rows per core
WS = 16.0    # fp8 weight prescale

EXP_SHIFT = 2.5
# fp8e4m3 bits of e^y are ~ round(8/ln2 * y + 56); y = s*SCALE - EXP_SHIFT
EXP_A = float(8.0 / np.log(2.0) * SCALE)
EXP_B = float(56.0 - 8.0 / np.log(2.0) * EXP_SHIFT)


def _build():
    nc = bacc.Bacc("TRN2", target_bir_lowering=False, debug=False,
                   num_devices=N_CORES)

    # All DRAM tensors are [128, X] with contiguous per-partition rows so
    # every DMA is one contiguous block.
    # xf8[p, (kc2, i, s)] = x[c = kc2*256 + i*128 + p, s]  (hf-rotated s),
    # split by contraction pair so pair-0 matmuls start sooner
    xf8a_d = nc.dram_tensor("xf8a", [128, 2048], F8,
                            kind="ExternalInput").ap()
    xf8b_d = nc.dram_tensor("xf8b", [128, 2048], F8,
                            kind="ExternalInput").ap()
    # xbf[p, (cc, sl)] = x[c = cc*128 + p, local half]  (residual read)
    xbf_d = nc.dram_tensor("xbf", [128, 2048], BF16,
                           kind="ExternalInput").ap()
    # wq8/wk8[p, (hp, pair, i, m)] = 16*W.T[pair*256+i*128+p, hp*128+m],
    # hp=0 split out so the first projections' weights arrive first
    wq8a_d = nc.dram_tensor("wq8a", [128, 512], F8,
                            kind="ExternalInput").ap()
    wq8b_d = nc.dram_tensor("wq8b", [128, 1536], F8,
                            kind="ExternalInput").ap()
    wk8a_d = nc.dram_tensor("wk8a", [128, 512], F8,
                            kind="ExternalInput").ap()
    wk8b_d = nc.dram_tensor("wk8b", [128, 1536], F8,
                            kind="ExternalInput").ap()
    # wv8[p, (pair, i, f)] = 16*Wv.T[pair*256+i*128+p, f]
    wv8_d = nc.dram_tensor("wv8", [128, 2048], F8, kind="ExternalInput").ap()
    # wo[p, (hd, m)] = Wo.T[hd*128+p, m]
    wo_d = nc.dram_tensor("wo", [128, 2048], BF16, kind="ExternalInput").ap()
    ident_d = nc.dram_tensor("ident", [128, 128], BF16,
                             kind="ExternalInput").ap()
    # bpack columns: bq (4 chunks) | bo' (4 chunks), bo' = bo + Wo @ bv
    bp_d = nc.dram_tensor("bpack", [128, 8], F32, kind="ExternalInput").ap()
    # out rows [cc*128 .. +128) = out chunk cc, bf16 (host upcasts)
    out_d = nc.dram_tensor("out", [C, SL], BF16, kind="ExternalOutput").ap()

    with tile.TileContext(nc) as tc, ExitStack() as ctx:
        cst = ctx.enter_context(tc.tile_pool(name="cst", bufs=1))
        rpool = ctx.enter_context(tc.tile_pool(name="rp", bufs=4))
        opool = ctx.enter_context(tc.tile_pool(name="op", bufs=4))
        # PSUM: one shared 4-deep rotation of [128,1024] tiles (8 banks)
        # serving scores, projections, attn@v pairs AND the out-proj.
        psc = ctx.enter_context(tc.tile_pool(name="psc", bufs=4,
                                             space="PSUM"))

        # ---- persistent SBUF tiles ----
        xf8a_sb = cst.tile([128, 2048], F8, tag="xf8a", name="xf8a")
        xf8b_sb = cst.tile([128, 2048], F8, tag="xf8b", name="xf8b")
        xbf_sb = cst.tile([128, 2048], BF16, tag="xbf", name="xbf")
        wq8a_sb = cst.tile([128, 512], F8, tag="wq8a", name="wq8a")
        wq8b_sb = cst.tile([128, 1536], F8, tag="wq8b", name="wq8b")
        wk8a_sb = cst.tile([128, 512], F8, tag="wk8a", name="wk8a")
        wk8b_sb = cst.tile([128, 1536], F8, tag="wk8b", name="wk8b")
        wv8_sb = cst.tile([128, 2048], F8, tag="wv8", name="wv8")
        wo_sb = cst.tile([128, 2048], BF16, tag="wo", name="wo")
        id_sb = cst.tile([128, 128], BF16, tag="id", name="id")
        bp_sb = cst.tile([128, 8], F32, tag="bp", name="bp")
        ebias_sb = cst.tile([128, 1], F32, tag="eb", name="eb")
        qT = [cst.tile([128, SL], BF16, tag=f"qT{i}", name=f"qT{i}")
              for i in range(4)]
        kT = [cst.tile([128, S], BF16, tag=f"kT{i}", name=f"kT{i}")
              for i in range(4)]
        # v8[j][p, (i, h, e)]: key chunks 2j+i; e in [ones(64) | v(64)]
        v8 = [cst.tile([128, 2048], F8, tag=f"v8_{j}", name=f"v8_{j}")
              for j in range(4)]
        # P[hp][jj][p, (kc, hi, n)] fp8: one tile per kc PAIR so an attn@v
        # matmul only waits on its own two exp writers (deps are
        # tile-granular)
        P = [[cst.tile([128, 2048], F8, tag=f"P{hp}_{jj}",
                       name=f"P{hp}_{jj}") for jj in range(4)]
             for hp in range(4)]
        resT = [cst.tile([128, SL], BF16, tag=f"resT{i}", name=f"resT{i}")
                for i in range(4)]

        def wqv(hp, pair):  # wq8 [128, 2, 128] DoubleRow view
            sb = wq8a_sb if hp == 0 else wq8b_sb
            g = sb[:].rearrange("p (hp pr i m) -> p hp pr i m",
                                hp=(1 if hp == 0 else 3), pr=2, i=2)
            return g[:, hp if hp == 0 else hp - 1, pair]

        def wkv(hp, pair):
            sb = wk8a_sb if hp == 0 else wk8b_sb
            g = sb[:].rearrange("p (hp pr i m) -> p hp pr i m",
                                hp=(1 if hp == 0 else 3), pr=2, i=2)
            return g[:, hp if hp == 0 else hp - 1, pair]

        def wvv(pair):  # wv8 [128, 2, 512]
            g = wv8_sb[:].rearrange("p (pr i f) -> p pr i f", pr=2, i=2)
            return g[:, pair]

        def xv(pair, n0, n1):  # xf8 [128, 2, n1-n0]
            sb = xf8a_sb if pair == 0 else xf8b_sb
            g = sb[:].rearrange("p (i s) -> p i s", i=2)
            return g[:, :, n0:n1]

        def v8w(j, h):  # v8 weights [128, 2, 128] for head h, kc pair j
            g = v8[j][:].rearrange("p (i h e) -> p i h e", i=2, h=8)
            return g[:, :, h, :]

        def pview(hp, hi, j):  # P [128, 2, 512] moving view for kc pair j
            g = P[hp][j][:].rearrange("p (kc hi n) -> p kc hi n",
                                      kc=2, hi=2)
            return g[:, :, hi, :]

        # ---- input DMAs: ALL on the sync ring in strict priority order
        # (DMA bandwidth is shared across rings; serializing behind the
        # critical first blocks guarantees their priority).
        # first blocks issue on TWO rings in parallel (the per-DMA issue
        # slot is ~650ns; the scalar engine is free this early)
        nc.sync.dma_start(xf8a_sb[:], xf8a_d[:])
        nc.scalar.dma_start(wq8a_sb[:], wq8a_d[:])
        nc.scalar.dma_start(xf8b_sb[:], xf8b_d[:])
        nc.sync.dma_start(wk8a_sb[:], wk8a_d[:])
        nc.sync.dma_start(bp_sb[:], bp_d[:])
        nc.sync.dma_start(wq8b_sb[:], wq8b_d[:])
        nc.sync.dma_start(wv8_sb[:], wv8_d[:])
        nc.sync.dma_start(wk8b_sb[:], wk8b_d[:])
        nc.sync.dma_start(wo_sb[:], wo_d[:])
        nc.sync.dma_start(id_sb[:], ident_d[:])
        nc.sync.dma_start(xbf_sb[:], xbf_d[:])
        # ebias + a dep-free dummy activation so the scalar ACT table
        # loads during startup instead of blocking the first exp.
        nc.gpsimd.memset(ebias_sb[:], EXP_B)
        junk_sb = cst.tile([128, 1], F32, tag="junk", name="junk")
        nc.scalar.activation(junk_sb[:], ebias_sb[:], AF.Exp, scale=1.0)
        # ones columns in every v8 tile (written once, gpsimd)
        for j in range(4):
            g = v8[j][:].rearrange("p (i h e) -> p i h e", i=2, h=8)
            nc.gpsimd.memset(g[:, :, :, 0:64], 1.0)

        # ---- emit units ----
        def emit_q(hp):
            # qT[hp] = (16 Wq[hp] @ xs_local^T)/16 + bq; hp=0's copy runs
            # on the DVE (idle during the first units)
            ps = psc.tile([128, 1024], F32, tag="sc", name="sc")[:, 0:512]
            for pair in range(2):
                nc.tensor.matmul(ps, wqv(hp, pair), xv(pair, 0, SL),
                                 start=(pair == 0), stop=(pair == 1),
                                 perf_mode=PM.DoubleRow)
            if hp == 0:
                nc.vector.tensor_scalar(qT[hp][:], ps,
                                        1.0 / WS, bp_sb[:, hp:hp + 1],
                                        op0=ALU.mult, op1=ALU.add)
            else:
                nc.scalar.activation(qT[hp][:], ps, AF.Identity,
                                     scale=1.0 / WS,
                                     bias=bp_sb[:, hp:hp + 1])

        def emit_k_ns(hp, ns, ps):
            # one 512-key half of kT[hp]; no bias (cancels in softmax)
            for pair in range(2):
                nc.tensor.matmul(
                    ps[:, ns * 512:(ns + 1) * 512],
                    wkv(hp, pair), xv(pair, ns * 512, (ns + 1) * 512),
                    start=(pair == 0), stop=(pair == 1),
                    perf_mode=PM.DoubleRow)
            if hp == 0 and ns == 1:
                # DVE is idle in the first units; parallelize the
                # startup kT chain
                nc.vector.tensor_scalar(kT[hp][:, 512:1024],
                                        ps[:, 512:1024],
                                        1.0 / WS, None, op0=ALU.mult)
            else:
                nc.scalar.activation(kT[hp][:, ns * 512:(ns + 1) * 512],
                                     ps[:, ns * 512:(ns + 1) * 512],
                                     AF.Copy, scale=1.0 / WS)

        def emit_k(hp):
            ps = psc.tile([128, 1024], F32, tag="sc", name="sc")
            emit_k_ns(hp, 0, ps)
            emit_k_ns(hp, 1, ps)

        def emit_sc(hp, kc, eng):
            # scoresT [128 keys of chunk kc, 512 q] for BOTH heads of the
            # pair in one [128,1024] tile; ONE exp op on engine `eng`.
            # Both engines write the identical Schraudolph fp8 bit
            # pattern (f32->uint8 conversion rounds and saturates low to
            # +0), so a softmax row never mixes exp methods.
            ps = psc.tile([128, 1024], F32, tag="sc", name="sc")
            for hi in range(2):
                base = hi * 64
                nc.tensor.matmul(
                    ps[:, hi * SL:(hi + 1) * SL],
                    kT[hp][base:base + 64, kc * 128:(kc + 1) * 128],
                    qT[hp][base:base + 64, :],
                    start=True, stop=True,
                )
            pdst = P[hp][kc // 2][:, (kc % 2) * 1024:
                                  (kc % 2) * 1024 + 1024].bitcast(U8)
            if eng == "s":
                nc.scalar.activation(pdst, ps[:], AF.Identity,
                                     scale=EXP_A, bias=ebias_sb[:])
            else:
                nc.vector.tensor_scalar(
                    pdst, ps[:],
                    EXP_A, EXP_B, op0=ALU.mult, op1=ALU.add,
                )

        def emit_v(j, eng):
            # v rows for key chunks 2j, 2j+1 -> fp8 with 1/16 rescale;
            # the cast runs on `eng` (split across scalar+vector)
            ps = psc.tile([128, 1024], F32, tag="sc", name="sc")
            for i in range(2):
                rc = 2 * j + i
                for pair in range(2):
                    nc.tensor.matmul(
                        ps[:, i * 512:(i + 1) * 512],
                        xv(pair, rc * 128, (rc + 1) * 128), wvv(pair),
                        start=(pair == 0), stop=(pair == 1),
                        perf_mode=PM.DoubleRow)
            g = v8[j][:].rearrange("p (i h e) -> p i h e", i=2, h=8)
            if eng == "s":
                nc.scalar.activation(
                    g[:, :, :, 64:128],
                    ps[:].rearrange("p (i h e) -> p i h e", i=2, h=8),
                    AF.Copy, scale=1.0 / WS)
            else:
                nc.vector.tensor_scalar(
                    g[:, :, :, 64:128],
                    ps[:].rearrange("p (i h e) -> p i h e", i=2, h=8),
                    1.0 / WS, None, op0=ALU.mult)

        def emit_av2(h, pr, j0=0, j1=4):
            # attn @ [ones|v]: psum rows 0:64 = den (x64), 64:128 = res;
            # kc pairs [j0, j1) of the accumulation group
            hp, hi = h // 2, h % 2
            for j in range(j0, j1):
                nc.tensor.matmul(pr, v8w(j, h), pview(hp, hi, j),
                                 start=(j == 0), stop=(j == 3),
                                 perf_mode=PM.DoubleRow)

        def emit_norm_pair(hp, prt):
            # prt [128, 1024]: head 2hp in cols 0:512, 2hp+1 in 512:1024;
            # rows 0:64 = den (x64 copies), 64:128 = res
            rc_t = rpool.tile([64, 1024], F32, tag="rc", name="rc")
            nc.vector.reciprocal_approx_fast(rc_t[:], prt[0:64, :])
            for hi in range(2):
                nc.vector.tensor_tensor(
                    resT[hp][hi * 64:(hi + 1) * 64, :],
                    prt[64:128, hi * 512:(hi + 1) * 512],
                    rc_t[:, hi * 512:(hi + 1) * 512], op=ALU.mult,
                )

        def emit_out_mm(cc, ps, hd, start, stop):
            nc.tensor.matmul(
                ps,
                wo_sb[:, hd * 512 + cc * 128:hd * 512 + (cc + 1) * 128],
                resT[hd][:],
                start=start, stop=stop,
            )

        def emit_out_res(cc, ps):
            # residual: I @ xs chunk cc (mid-group)
            nc.tensor.matmul(ps, id_sb[:],
                             xbf_sb[:, cc * 512:(cc + 1) * 512],
                             start=False, stop=False)

        def emit_out_epi(cc, ps):
            # psum + bo' -> bf16; split across scalar/vector and both DMA
            # rings so the four epilogues pipeline at the tail
            ot = opool.tile([128, SL], BF16, tag="ob", name="ob")
            if cc % 2 == 0:
                nc.scalar.activation(ot[:], ps, AF.Identity,
                                     scale=1.0,
                                     bias=bp_sb[:, 4 + cc:5 + cc])
            else:
                nc.vector.tensor_scalar(ot[:], ps,
                                        bp_sb[:, 4 + cc:5 + cc], None,
                                        op0=ALU.add)
            q = nc.sync if cc < 2 else nc.scalar
            q.dma_start(out_d[cc * 128:(cc + 1) * 128, :], ot[:])

        # ---- woven emission schedule ----
        # Scores tiles stream to the two exp engines; every sc unit is
        # paired with independent PE filler (projections, v units, attn@v
        # pairs, out-proj prestarts) so the PE stays continuously busy
        # (idle resets the 2.4GHz p-state to 1.2GHz for 3us). v casts run
        # first (attn@v reads all four v8 tiles); attn@v pairs weave in
        # once their P tiles are a couple of units past their scores.
        av_pr = {}

        def av_pair_mm(hp, j1=4):
            prt = psc.tile([128, 1024], F32, tag="sc", name="sc")
            av_pr[hp] = prt
            emit_av2(2 * hp, prt[:, 0:512], 0, j1)

        def av_pair_mm2(hp):
            emit_av2(2 * hp + 1, av_pr[hp][:, 512:1024])

        def av_pair_mm2_31():
            emit_av2(7, av_pr[3][:, 512:1024], 0, 2)

        pso = {}

        def out_prestart(cc):
            ps = pso[cc]
            emit_out_mm(cc, ps, 1, True, False)
            emit_out_mm(cc, ps, 0, False, False)
            emit_out_res(cc, ps)

        # PE clock warmup: the PE idles ~7-10.7us waiting for input DMAs
        # and then pays the 1.2GHz p-state ramp on the first ~3us of real
        # matmuls. These dummy matmuls have NO input dependencies (warm_sb
        # is never written -> garbage data, result discarded when the
        # psum slot's next real user starts with start=True), so they
        # fire at preamble end and hand the real matmuls a 2.4GHz PE.
        warm_sb = cst.tile([128, 512], BF16, tag="warm", name="warm")
        nc.vector.memset(warm_sb[:], 0.0)  # DVE is idle until ~12.8us
        wps = psc.tile([128, 1024], F32, tag="sc", name="sc")
        for i in range(7):
            nc.tensor.matmul(wps[:, 0:512], warm_sb[:, 0:128],
                             warm_sb[:], start=True, stop=True)

        # prologue: q(0) and the FIRST half of k(0), with their pair-0
        # matmuls interleaved before the pair-1 ones so the PE has work
        # (k's pair-0 needs only the early wk8a+xf8a) while the xf8b DMA
        # is still in flight. The k ns1 half is deferred past the first
        # scores unit's emission so sc(0,0) doesn't wait on the (late,
        # vector-queued) ns1 copy (readers only wait on prior writers).
        qps0 = psc.tile([128, 1024], F32, tag="sc", name="sc")[:, 0:512]
        kps0 = psc.tile([128, 1024], F32, tag="sc", name="sc")
        nc.tensor.matmul(qps0, wqv(0, 0), xv(0, 0, SL),
                         start=True, stop=False, perf_mode=PM.DoubleRow)
        nc.tensor.matmul(kps0[:, 0:512], wkv(0, 0), xv(0, 0, 512),
                         start=True, stop=False, perf_mode=PM.DoubleRow)
        nc.tensor.matmul(qps0, wqv(0, 1), xv(1, 0, SL),
                         start=False, stop=True, perf_mode=PM.DoubleRow)
        nc.tensor.matmul(kps0[:, 0:512], wkv(0, 1), xv(1, 0, 512),
                         start=False, stop=True, perf_mode=PM.DoubleRow)
        nc.vector.tensor_scalar(qT[0][:], qps0, 1.0 / WS, bp_sb[:, 0:1],
                                op0=ALU.mult, op1=ALU.add)
        nc.scalar.activation(kT[0][:, 0:512], kps0[:, 0:512],
                             AF.Copy, scale=1.0 / WS)

        def alloc_pso01():
            pso01 = psc.tile([128, 1024], F32, tag="sc", name="sc")
            pso[0] = pso01[:, 0:512]
            pso[1] = pso01[:, 512:1024]

        # filler per 1-based unit index (32 units of (hp, kc))
        fillers = {
            1: lambda: emit_k_ns(0, 1, kps0),
            2: lambda: emit_v(0, "v"),
            4: lambda: emit_v(1, "s"),
            5: lambda: emit_q(1),
            6: lambda: emit_k(1),
            8: lambda: emit_v(2, "v"),
            9: lambda: emit_v(3, "s"),
            12: lambda: av_pair_mm(0),
            13: lambda: emit_q(2),
            14: lambda: emit_k(2),
            15: lambda: av_pair_mm2(0),
            16: lambda: emit_norm_pair(0, av_pr[0][:]),
            20: lambda: av_pair_mm(1),
            21: lambda: emit_q(3),
            22: lambda: emit_k(3),
            23: lambda: av_pair_mm2(1),
            24: lambda: emit_norm_pair(1, av_pr[1][:]),
            26: lambda: av_pair_mm(2),
            28: lambda: (alloc_pso01(), out_prestart(0)),
            29: lambda: av_pair_mm2(2),
            30: lambda: (out_prestart(1),
                         emit_norm_pair(2, av_pr[2][:])),
            31: lambda: av_pair_mm(3, j1=2),
            32: lambda: av_pair_mm2_31(),
        }
        # exp engine per unit: 18 scalar / 14 vector, spread evenly
        u = 0
        for hp in range(4):
            for kc in range(8):
                u += 1
                eng = "s" if (u * 18) // 32 > ((u - 1) * 18) // 32 else "v"
                emit_sc(hp, kc, eng)
                if u in fillers:
                    fillers[u]()

        # tail: last attn@v pair + remaining out-proj; the cc2/cc3
        # prestarts run first so the PE has work while the last P tiles
        # finish their exp; hd3 matmuls wait only on resT[3]; epilogues
        # run after ALL matmuls so no epi read blocks a later matmul's
        # write to the shared psum tile.
        pso23 = psc.tile([128, 1024], F32, tag="sc", name="sc")
        pso[2] = pso23[:, 0:512]
        pso[3] = pso23[:, 512:1024]
        out_prestart(2)
        out_prestart(3)
        # finish the last pair's accumulation (kc pairs 2,3)
        emit_av2(6, av_pr[3][:, 0:512], 2, 4)
        emit_av2(7, av_pr[3][:, 512:1024], 2, 4)
        # per-head norm on the last pair: both recips first (head 7's
        # doesn't queue behind head 6's multiply)
        rc_l = []
        for hi in range(2):
            rc_t = rpool.tile([64, 512], F32, tag=f"rcl{hi}",
                              name=f"rcl{hi}")
            nc.vector.reciprocal_approx_fast(
                rc_t[:], av_pr[3][0:64, hi * 512:(hi + 1) * 512])
            rc_l.append(rc_t)
        for hi in range(2):
            nc.vector.tensor_tensor(
                resT[3][hi * 64:(hi + 1) * 64, :],
                av_pr[3][64:128, hi * 512:(hi + 1) * 512],
                rc_l[hi][:], op=ALU.mult,
            )
        for cc in range(4):
            emit_out_mm(cc, pso[cc], 2, False, False)
        # epilogues fire as soon as their shared psum tile's LAST writer
        # (hd3 of the other cc half) is emitted
        for cc in range(2):
            emit_out_mm(cc, pso[cc], 3, False, True)
        emit_out_epi(0, pso[0])
        emit_out_epi(1, pso[1])
        for cc in range(2, 4):
            emit_out_mm(cc, pso[cc], 3, False, True)
        emit_out_epi(2, pso[2])
        emit_out_epi(3, pso[3])

    nc.compile()
    return nc


_NC_CACHE = None


def _get_nc():
    global _NC_CACHE
    if _NC_CACHE is None:
        _NC_CACHE = _build()
    return _NC_CACHE


def _prep_inputs(x, Wp, bp, Wo, bo):
    """Host-side reshape/reorder of weights; returns per-core input maps."""
    x = np.ascontiguousarray(x, dtype=np.float32)
    Wp = np.asarray(Wp, dtype=np.float32)
    bp = np.asarray(bp, dtype=np.float32)
    Wo = np.asarray(Wo, dtype=np.float32)
    bo = np.asarray(bo, dtype=np.float32)

    # Wp rows per head h: [h*192, h*192+64) = q, +64..128 = k, +128..192 = v
    Wp3 = Wp.reshape(NH, 3, DK, C)
    Wq = Wp3[:, 0].reshape(NH * DK, C)
    Wk = Wp3[:, 1].reshape(NH * DK, C)
    Wv = Wp3[:, 2].reshape(NH * DK, C)
    bp3 = bp.reshape(NH, 3, DK)
    bq = bp3[:, 0].reshape(-1)
    bv = bp3[:, 2].reshape(-1)
    # fold the v bias into the output projection bias (attn rows sum to 1)
    bo_eff = bo + Wo @ bv

    def pack_dr_w(WT, width):
        # WT [C, width] -> [128, (pair, i, width)]: 16*WT fp8 DoubleRow
        w = (WT * WS).astype(NP_F8)
        out = np.empty((128, 2, 2, width), dtype=NP_F8)
        for pair in range(2):
            for i in range(2):
                out[:, pair, i, :] = w[pair * 256 + i * 128:
                                       pair * 256 + i * 128 + 128, :]
        return np.ascontiguousarray(out.reshape(128, 4 * width))

    def pack_dr_whp(WT):
        # WT [C, 512] -> [128, (hp, pair, i, 128)]
        w = (WT * WS).astype(NP_F8)
        out = np.empty((128, 4, 2, 2, 128), dtype=NP_F8)
        for hp in range(4):
            for pair in range(2):
                for i in range(2):
                    out[:, hp, pair, i, :] = \
                        w[pair * 256 + i * 128:pair * 256 + i * 128 + 128,
                          hp * 128:(hp + 1) * 128]
        return np.ascontiguousarray(out.reshape(128, 2048))

    bpack = np.concatenate(
        [bq.reshape(4, 128).T, bo_eff.reshape(4, 128).T], axis=1)

    wq8 = pack_dr_whp(Wq.T)
    wk8 = pack_dr_whp(Wk.T)
    shared = {
        "wq8a": np.ascontiguousarray(wq8[:, 0:512]),
        "wq8b": np.ascontiguousarray(wq8[:, 512:2048]),
        "wk8a": np.ascontiguousarray(wk8[:, 0:512]),
        "wk8b": np.ascontiguousarray(wk8[:, 512:2048]),
        "wv8": pack_dr_w(Wv.T, 512),
        "wo": np.ascontiguousarray(Wo.T.reshape(4, 128, 512)
                                   .transpose(1, 0, 2).reshape(128, 2048)
                                   .astype(NP_BF16)),
        "ident": np.ascontiguousarray(np.eye(128, dtype=NP_BF16)),
        "bpack": np.ascontiguousarray(bpack.astype(np.float32)),
    }

    in_maps = []
    for c in range(N_CORES):
        b, hf = c // 2, c % 2
        xbc = x[b].reshape(C, S)
        if hf == 0:
            xs = xbc
        else:
            xs = np.concatenate([xbc[:, SL:], xbc[:, :SL]], axis=1)
        m = dict(shared)
        # xf8: [128, (i, s)] per contraction pair
        xf = xs.reshape(4, 128, S).astype(NP_F8)  # chunk-major
        xf = xf.transpose(1, 0, 2).reshape(128, 4096)
        m["xf8a"] = np.ascontiguousarray(xf[:, 0:2048])
        m["xf8b"] = np.ascontiguousarray(xf[:, 2048:4096])
        # xbf: [128, (cc, sl)] local half only
        xl = xs[:, 0:SL].reshape(4, 128, SL).astype(NP_BF16)
        m["xbf"] = np.ascontiguousarray(
            xl.transpose(1, 0, 2).reshape(128, 2048))
        in_maps.append(m)
    return in_maps


def _unshard(results):
    out = np.empty((B, C, S), dtype=np.float32)
    for c in range(N_CORES):
        b, hf = c // 2, c % 2
        out[b][:, hf * SL:(hf + 1) * SL] = \
            results[c]["out"].astype(np.float32)  # [C, SL]
    H = int(np.sqrt(S))
    return out.reshape(B, C, H, H)


def kernel(x, Wp, bp, Wo, bo):
    nc = _get_nc()
    in_maps = _prep_inputs(x, Wp, bp, Wo, bo)
    res = run_bass_kernel_spmd(nc, in_maps, list(range(N_CORES)))
    return _unshard(res.results)
